# revision 1
# baseline (speedup 1.0000x reference)
"""ChebConv (K=3, two layers + softmax) GNN kernel for 8 Trainium2 NeuronCores.

Strategy (dst-node sharding, graph replicated):
  - Nodes are split into 8 contiguous shards (one per core); each core owns the
    edges whose *destination* lies in its shard.
  - Host preprocessing sorts edges by dst, groups them per 128-dst block, and
    pads each block's edge list to a whole number of 128-edge chunks.  Chunk
    counts per block are maxed across cores so a single SPMD program serves
    all 8 cores (padding edges have w=0 and gather table row 0).
  - SpMM per chunk: a 128-row indirect DMA gathers x[src[e]] (256B rows) from
    a replicated table in DRAM; the DVE builds S_T[e, d] = (iota==dst_local)*w
    in one dual-op tensor_scalar; the PE accumulates X_g.T @ S_T into PSUM,
    yielding feature-major [64, 128] output blocks that feed the dense W_k
    matmuls directly (contraction over features needs features on partitions).
  - T2 = 2*A*T1 - T0 stays on chip (the factor 2 is folded into a second edge
    weight array); T1-type outputs are PE-transposed back to row-major and
    stored as the next gather table.  Feature-major copies of x/T1/h/T3 blocks
    bounce through DRAM between phases (bulk DMA, cheap next to the gathers).
  - Tables computed on device (T1, h, T3) are exchanged with AllGather
    collectives (3.2MB per core each) between SpMM phases.
  - Bias adds are folded into ACT activations (Relu for layer 1, Identity for
    layer 2); softmax runs per 128-node block after a final PE transpose.
"""

import os

import numpy as np

import concourse.bass as bass
import concourse.mybir as mybir
import concourse.tile as tile
from concourse import bacc
from concourse.bass import IndirectOffsetOnAxis
from concourse.bass_utils import run_bass_kernel_spmd

NCORES = 8
P = 128
NQ = 4  # SWDGE queues used round-robin for the gather stream

F32 = mybir.dt.float32
I32 = mybir.dt.int32
ALU = mybir.AluOpType
ACTF = mybir.ActivationFunctionType


def _preprocess(x, edge_index):
    N, F = x.shape
    assert N % NCORES == 0
    PN = N // NCORES
    NB = (PN + P - 1) // P
    PNP = NB * P
    TROWS = NCORES * PNP

    src = edge_index[0].astype(np.int64)
    dst = edge_index[1].astype(np.int64)
    keep = src != dst
    deg = np.bincount(src[keep], minlength=N).astype(np.float32)
    dis = np.where(deg > 0, 1.0 / np.sqrt(np.maximum(deg, 1.0)), 0.0).astype(
        np.float32
    )
    w = np.where(keep, -dis[src] * dis[dst], 0.0).astype(np.float32)

    order = np.argsort(dst, kind="stable")
    s_s, s_d, s_w = src[order], dst[order], w[order]
    core_of = s_d // PN
    dl = s_d % PN
    blk = dl // P
    dstloc = (dl % P).astype(np.float32)

    # gather-table row of each edge's source node, and its int16 sub-table
    trow = ((s_s // PN) * PNP + (s_s % PN)).astype(np.int64)
    QROWS = TROWS // 4
    assert QROWS <= 32768
    quad = trow // QROWS

    # bucket edges by (core, block, src-quadrant); order within bucket free
    cnt = np.zeros((NCORES, NB, 4), np.int64)
    np.add.at(cnt, (core_of, blk, quad), 1)
    cbq = -(-cnt.max(axis=0) // P)            # [NB, 4] chunks per bucket
    cbq[:, 0] = np.maximum(cbq[:, 0], 1)      # each block needs >= 1 chunk
    cb = cbq.sum(axis=1)                      # chunks per block
    CT = int(cb.sum())
    cbase_q = np.zeros((NB, 4), np.int64)
    flat = cbq.ravel()
    off = np.zeros(NB * 4, np.int64)
    off[1:] = np.cumsum(flat)[:-1]
    cbase_q = off.reshape(NB, 4)

    key = (core_of * NB + blk) * 4 + quad
    korder = np.argsort(key, kind="stable")
    starts = np.zeros(NCORES * NB * 4 + 1, np.int64)
    starts[1:] = np.cumsum(cnt.ravel())
    starts = starts[:-1].reshape(NCORES, NB, 4)

    t_r, d_l, w_s = trow[korder], dstloc[korder], s_w[korder]
    eidx16 = np.zeros((NCORES, P, CT * 8), np.int16)
    edst = np.zeros((NCORES, P, CT), np.float32)
    ew = np.zeros((NCORES, P, CT), np.float32)
    base_ptr = 0
    for c in range(NCORES):
        for b in range(NB):
            for q in range(4):
                n = int(cnt[c, b, q])
                s0 = int(starts[c, b, q])
                if n:
                    js = np.arange(n)
                    cols = cbase_q[b, q] + js // P
                    parts = js % P
                    edst[c, parts, cols] = d_l[s0 : s0 + n]
                    ew[c, parts, cols] = w_s[s0 : s0 + n]
                    # int16 local rows, wrapped 16-wide, padded to chunk size
                    npad = int(cbq[b, q]) * P
                    loc = np.zeros(npad, np.int64)
                    loc[:n] = t_r[s0 : s0 + n] - q * QROWS
                    wrap = loc.reshape(-1, 16).T.astype(np.int16)  # [16, npad/16]
                    c0 = cbase_q[b, q] * 8
                    eidx16[c, :, c0 : c0 + npad // 16] = np.tile(wrap, (8, 1))
                elif int(cbq[b, q]):
                    pass  # all-padding chunk: zeros already there

    # replicated padded gather table for x; per-core feature-major x blocks
    x_rm = np.zeros((TROWS, F), np.float32)
    xfm = np.zeros((NCORES, NB, F, P), np.float32)
    for c in range(NCORES):
        x_rm[c * PNP : c * PNP + PN] = x[c * PN : (c + 1) * PN]
        xpad = np.zeros((PNP, F), np.float32)
        xpad[:PN] = x[c * PN : (c + 1) * PN]
        xfm[c] = xpad.reshape(NB, P, F).transpose(0, 2, 1)

    return dict(
        PN=PN, NB=NB, PNP=PNP, TROWS=TROWS, F=F, cb=cb.tolist(),
        cbq=cbq.tolist(), cbase_q=cbase_q.tolist(), CT=CT,
        eidx16=eidx16, edst=edst, edstn=-edst, ew=ew, x_rm=x_rm, xfm=xfm,
    )


def _build(meta, K, HID, NCLS):
    NB, PNP, TROWS, F, CT = (
        meta["NB"], meta["PNP"], meta["TROWS"], meta["F"], meta["CT"]
    )
    cb = meta["cb"]
    cbq, cbase_q = meta["cbq"], meta["cbase_q"]
    QROWS = TROWS // 4

    nc = bacc.Bacc(
        "TRN2", target_bir_lowering=False, debug=False,
        num_devices=NCORES, num_swdge_queues=NQ,
    )
    d_xrm = nc.dram_tensor("x_rm", [TROWS, F], F32, kind="ExternalInput")
    d_xfm = nc.dram_tensor("xfm", [NB, F, P], F32, kind="ExternalInput")
    d_W1 = nc.dram_tensor("W1", [K, F, HID], F32, kind="ExternalInput")
    d_W2 = nc.dram_tensor("W2", [K, HID, NCLS], F32, kind="ExternalInput")
    d_b1 = nc.dram_tensor("b1", [HID, 1], F32, kind="ExternalInput")
    d_b2 = nc.dram_tensor("b2", [NCLS, 1], F32, kind="ExternalInput")
    d_iota = nc.dram_tensor("iota", [P, P], F32, kind="ExternalInput")
    d_ident = nc.dram_tensor("ident", [P, P], F32, kind="ExternalInput")
    d_eidx = nc.dram_tensor("eidx", [P, CT * 8], mybir.dt.int16, kind="ExternalInput")
    d_edst = nc.dram_tensor("edst", [P, CT], F32, kind="ExternalInput")
    d_edstn = nc.dram_tensor("edstn", [P, CT], F32, kind="ExternalInput")
    d_ew = nc.dram_tensor("ew", [P, CT], F32, kind="ExternalInput")
    d_out = nc.dram_tensor("out", [PNP, NCLS], F32, kind="ExternalOutput")

    d_t1loc = nc.dram_tensor("t1loc", [PNP, F], F32)
    d_hloc = nc.dram_tensor("hloc", [PNP, F], F32)
    d_t3loc = nc.dram_tensor("t3loc", [PNP, F], F32)
    d_t1fm = nc.dram_tensor("t1fm", [NB, F, P], F32)
    d_hfm = nc.dram_tensor("hfm", [NB, F, P], F32)
    d_t3fm = nc.dram_tensor("t3fm", [NB, F, P], F32)
    d_t1full = nc.dram_tensor("t1full", [TROWS, F], F32, addr_space="Shared")
    d_hfull = nc.dram_tensor("hfull", [TROWS, F], F32, addr_space="Shared")
    d_t3full = nc.dram_tensor("t3full", [TROWS, F], F32, addr_space="Shared")

    groups = [list(range(NCORES))]
    qcounter = [0]

    with tile.TileContext(nc) as tc:
        with (
            tc.tile_pool(name="const", bufs=1) as constp,
            tc.tile_pool(name="big", bufs=1) as bigp,
            tc.tile_pool(name="xg", bufs=12) as xgp,
            tc.tile_pool(name="st", bufs=6) as stp,
            tc.tile_pool(name="fm", bufs=3) as fmp,
            tc.tile_pool(name="tmp", bufs=4) as tmpp,
            tc.tile_pool(name="sm", bufs=4) as smp,
            tc.tile_pool(name="pseg", bufs=2, space="PSUM") as psegp,
            tc.tile_pool(name="pden", bufs=2, space="PSUM") as pdenp,
            tc.tile_pool(name="ptr", bufs=2, space="PSUM") as ptrp,
        ):
            # ---- constants / resident tiles ----
            iota = constp.tile([P, P], F32, tag="iota")
            ident = constp.tile([P, P], F32, tag="ident")
            w1k = [constp.tile([F, HID], F32, tag=f"w1_{k}", name=f"w1_{k}")
                   for k in range(K)]
            w2k = [constp.tile([HID, NCLS], F32, tag=f"w2_{k}", name=f"w2_{k}")
                   for k in range(K)]
            b1c = constp.tile([HID, 1], F32, tag="b1c")
            b2c = constp.tile([NCLS, 1], F32, tag="b2c")
            eidx = bigp.tile([P, CT * 8], mybir.dt.int16, tag="eidx")
            edst = bigp.tile([P, CT], F32, tag="edst")
            edstn = bigp.tile([P, CT], F32, tag="edstn")
            ew1 = bigp.tile([P, CT], F32, tag="ew1")
            ew2 = bigp.tile([P, CT], F32, tag="ew2")

            nc.sync.dma_start(iota[:], d_iota[:])
            nc.sync.dma_start(ident[:], d_ident[:])
            for k in range(K):
                nc.sync.dma_start(w1k[k][:], d_W1[k])
                nc.sync.dma_start(w2k[k][:], d_W2[k])
            nc.sync.dma_start(b1c[:], d_b1[:])
            nc.sync.dma_start(b2c[:], d_b2[:])
            nc.sync.dma_start(eidx[:], d_eidx[:])
            nc.sync.dma_start(edst[:], d_edst[:])
            nc.sync.dma_start(edstn[:], d_edstn[:])
            nc.sync.dma_start(ew1[:], d_ew[:])
            nc.vector.tensor_scalar(
                out=ew2[:], in0=ew1[:], scalar1=2.0, scalar2=None, op0=ALU.mult
            )

            def seg_block(b, table, wcols):
                """SpMM for dst-block b via dma_gather over the four int16
                sub-tables; returns PSUM tile [F, 128]."""
                seg = psegp.tile([F, P], F32, tag="seg", name=f"seg{b}")
                total = cb[b]
                done = 0
                for q in range(4):
                    kq = cbq[b][q]
                    cq = 0
                    while cq < kq:
                        k = min(8, kq - cq)
                        c0 = cbase_q[b][q] + cq
                        xg = xgp.tile([P, 8, F], F32, tag="xg",
                                      name=f"xg{b}_{q}_{cq}")
                        nc.gpsimd.dma_gather(
                            out_ap=xg[:, :k, :],
                            in_ap=table[q * QROWS : (q + 1) * QROWS, :],
                            idxs_ap=eidx[:, c0 * 8 : (c0 + k) * 8],
                            num_idxs=k * P, num_idxs_reg=k * P,
                            elem_size=F,
                            queue_num=qcounter[0] % NQ,
                        )
                        qcounter[0] += 1
                        for j in range(k):
                            c = c0 + j
                            st = stp.tile([P, P], F32, tag="st",
                                          name=f"st{b}_{q}_{cq}_{j}")
                            if qcounter[0] % 4 == 0:
                                # DVE path: one-hot carries the edge weight
                                nc.vector.tensor_scalar(
                                    out=st[:], in0=iota[:],
                                    scalar1=edst[:, c : c + 1],
                                    scalar2=wcols[:, c : c + 1],
                                    op0=ALU.is_equal, op1=ALU.mult,
                                )
                                lhs = xg[:, j, :]
                            else:
                                # ACT path: pure 0/1 one-hot via
                                # relu(1 - (iota - dst)^2); w applied to the
                                # gathered rows on DVE ([128,64], half the
                                # elements of the one-hot build).
                                sq = stp.tile([P, P], F32, tag="sq",
                                              name=f"sq{b}_{q}_{cq}_{j}")
                                nc.scalar.activation(
                                    sq[:], iota[:], ACTF.Square,
                                    bias=edstn[:, c : c + 1], scale=1.0,
                                )
                                nc.scalar.activation(
                                    st[:], sq[:], ACTF.Relu,
                                    bias=1.0, scale=-1.0,
                                )
                                xgw = stp.tile([P, F], F32, tag="xgw",
                                               name=f"xgw{b}_{q}_{cq}_{j}")
                                nc.vector.tensor_scalar(
                                    out=xgw[:], in0=xg[:, j, :],
                                    scalar1=wcols[:, c : c + 1], scalar2=None,
                                    op0=ALU.mult,
                                )
                                lhs = xgw[:]
                            nc.tensor.matmul(
                                seg[:], lhsT=lhs, rhs=st[:],
                                start=(done == 0), stop=(done == total - 1),
                            )
                            done += 1
                        cq += k
                return seg

            def store_rowmajor(b, fm_tile, dest):
                """Transpose a feature-major [F,128] SBUF tile and store it
                row-major [128,F] into DRAM table `dest`."""
                tr = ptrp.tile([P, F], F32, tag="tr", name=f"tr{b}")
                nc.tensor.transpose(tr[:], fm_tile, ident[:F, :F])
                rm = tmpp.tile([P, F], F32, tag="rm", name=f"rm{b}")
                nc.scalar.copy(rm[:], tr[:])
                nc.sync.dma_start(dest[b * P : (b + 1) * P, :], rm[:])

            # ---- phase 1: T1 = A @ x ----
            for b in range(NB):
                seg = seg_block(b, d_xrm, ew1)
                t1b = fmp.tile([F, P], F32, tag="fmt", name=f"t1b{b}")
                nc.vector.tensor_copy(t1b[:], seg[:])
                nc.sync.dma_start(d_t1fm[b], t1b[:])
                store_rowmajor(b, t1b[:], d_t1loc)
            nc.gpsimd.collective_compute(
                "AllGather", ALU.bypass, replica_groups=groups,
                ins=[d_t1loc[:]], outs=[d_t1full[:]],
            )

            # ---- phase 2: T2 = 2 A T1 - x;  h = relu(sum_k Tk @ W1k + b1) ----
            for b in range(NB):
                seg = seg_block(b, d_t1full, ew2)
                xfmb = fmp.tile([F, P], F32, tag="fmt", name=f"xfmb{b}")
                nc.sync.dma_start(xfmb[:], d_xfm[b])
                t1fmb = fmp.tile([F, P], F32, tag="fmt", name=f"t1fmb{b}")
                nc.sync.dma_start(t1fmb[:], d_t1fm[b])
                tx2 = tmpp.tile([F, P], F32, tag="tx2", name=f"tx2_{b}")
                nc.vector.tensor_tensor(
                    out=tx2[:], in0=seg[:], in1=xfmb[:], op=ALU.subtract
                )
                o1 = pdenp.tile([HID, P], F32, tag="oden", name=f"o1_{b}")
                nc.tensor.matmul(o1[:], lhsT=w1k[0][:], rhs=xfmb[:],
                                 start=True, stop=False)
                nc.tensor.matmul(o1[:], lhsT=w1k[1][:], rhs=t1fmb[:],
                                 start=False, stop=False)
                nc.tensor.matmul(o1[:], lhsT=w1k[2][:], rhs=tx2[:],
                                 start=False, stop=True)
                hb = fmp.tile([F, P], F32, tag="fmt", name=f"hb{b}")
                nc.scalar.activation(hb[:], o1[:], ACTF.Relu, bias=b1c[:])
                nc.sync.dma_start(d_hfm[b], hb[:])
                store_rowmajor(b, hb[:], d_hloc)
            nc.gpsimd.collective_compute(
                "AllGather", ALU.bypass, replica_groups=groups,
                ins=[d_hloc[:]], outs=[d_hfull[:]],
            )

            # ---- phase 3: T3 = A @ h ----
            for b in range(NB):
                seg = seg_block(b, d_hfull, ew1)
                t3b = fmp.tile([F, P], F32, tag="fmt", name=f"t3b{b}")
                nc.vector.tensor_copy(t3b[:], seg[:])
                nc.sync.dma_start(d_t3fm[b], t3b[:])
                store_rowmajor(b, t3b[:], d_t3loc)
            nc.gpsimd.collective_compute(
                "AllGather", ALU.bypass, replica_groups=groups,
                ins=[d_t3loc[:]], outs=[d_t3full[:]],
            )

            # ---- phase 4: out = softmax(sum_k Tk @ W2k + b2) ----
            for b in range(NB):
                seg = seg_block(b, d_t3full, ew2)
                hfmb = fmp.tile([F, P], F32, tag="fmt", name=f"hfmb{b}")
                nc.sync.dma_start(hfmb[:], d_hfm[b])
                t3fmb = fmp.tile([F, P], F32, tag="fmt", name=f"t3fmb{b}")
                nc.sync.dma_start(t3fmb[:], d_t3fm[b])
                th2 = tmpp.tile([F, P], F32, tag="tx2", name=f"th2_{b}")
                nc.vector.tensor_tensor(
                    out=th2[:], in0=seg[:], in1=hfmb[:], op=ALU.subtract
                )
                o2 = pdenp.tile([NCLS, P], F32, tag="oden", name=f"o2_{b}")
                nc.tensor.matmul(o2[:], lhsT=w2k[0][:], rhs=hfmb[:],
                                 start=True, stop=False)
                nc.tensor.matmul(o2[:], lhsT=w2k[1][:], rhs=t3fmb[:],
                                 start=False, stop=False)
                nc.tensor.matmul(o2[:], lhsT=w2k[2][:], rhs=th2[:],
                                 start=False, stop=True)
                o2b = tmpp.tile([NCLS, P], F32, tag="o2b", name=f"o2b{b}")
                nc.scalar.activation(o2b[:], o2[:], ACTF.Identity, bias=b2c[:])
                tr2 = ptrp.tile([P, NCLS], F32, tag="tr", name=f"tr2_{b}")
                nc.tensor.transpose(tr2[:], o2b[:], ident[:NCLS, :NCLS])
                o2t = smp.tile([P, NCLS], F32, tag="o2t", name=f"o2t{b}")
                nc.vector.tensor_copy(o2t[:], tr2[:])
                negm = smp.tile([P, 1], F32, tag="negm", name=f"negm{b}")
                nc.vector.tensor_reduce(
                    negm[:], o2t[:], axis=mybir.AxisListType.X,
                    op=ALU.max, negate=True,
                )
                ex = smp.tile([P, NCLS], F32, tag="ex", name=f"ex{b}")
                nc.scalar.activation(ex[:], o2t[:], ACTF.Exp, bias=negm[:])
                ssum = smp.tile([P, 1], F32, tag="ssum", name=f"ssum{b}")
                nc.vector.tensor_reduce(
                    ssum[:], ex[:], axis=mybir.AxisListType.X, op=ALU.add
                )
                rcp = smp.tile([P, 1], F32, tag="rcp", name=f"rcp{b}")
                nc.vector.reciprocal(rcp[:], ssum[:])
                res = smp.tile([P, NCLS], F32, tag="res", name=f"res{b}")
                nc.vector.tensor_scalar(
                    out=res[:], in0=ex[:], scalar1=rcp[:, :1], scalar2=None,
                    op0=ALU.mult,
                )
                nc.sync.dma_start(d_out[b * P : (b + 1) * P, :], res[:])

    nc.compile()
    return nc


def kernel(x, edge_index, W1, b1, W2, b2, _backend="hw"):
    x = np.asarray(x, dtype=np.float32)
    edge_index = np.asarray(edge_index, dtype=np.int32)
    W1 = np.asarray(W1, dtype=np.float32)
    b1 = np.asarray(b1, dtype=np.float32)
    W2 = np.asarray(W2, dtype=np.float32)
    b2 = np.asarray(b2, dtype=np.float32)
    K, F, HID = W1.shape
    NCLS = W2.shape[2]

    meta = _preprocess(x, edge_index)
    nc = _build(meta, K, HID, NCLS)

    iota = np.tile(np.arange(P, dtype=np.float32), (P, 1))
    ident = np.eye(P, dtype=np.float32)
    in_maps = []
    for c in range(NCORES):
        in_maps.append({
            "x_rm": meta["x_rm"], "xfm": meta["xfm"][c],
            "W1": W1, "W2": W2,
            "b1": b1.reshape(-1, 1).astype(np.float32),
            "b2": b2.reshape(-1, 1).astype(np.float32),
            "iota": iota, "ident": ident,
            "eidx": meta["eidx16"][c], "edst": meta["edst"][c], "edstn": meta["edstn"][c],
            "ew": meta["ew"][c],
        })

    PN = meta["PN"]
    if _backend == "sim":
        from concourse.bass_interp import MultiCoreSim

        sim = MultiCoreSim(nc, num_cores=NCORES)
        for c in range(NCORES):
            for name, arr in in_maps[c].items():
                sim.cores[c].tensor(name)[:] = arr
        sim.simulate()
        outs = [np.array(sim.cores[c].tensor("out"))[:PN] for c in range(NCORES)]
        kernel.last_result = None
        return np.concatenate(outs, axis=0)

    trace = bool(os.environ.get("BASS_TRACE"))
    res = run_bass_kernel_spmd(
        nc, in_maps, core_ids=list(range(NCORES)), trace=trace
    )
    kernel.last_result = res
    return np.concatenate(
        [res.results[c]["out"][:PN] for c in range(NCORES)], axis=0
    )

